# revision 7
# baseline (speedup 1.0000x reference)
"""TRN2 Bass kernel: masked-centroid squared distances (8 NeuronCores, SPMD).

Reference computation (fp32):
    C = U^T X / B                          [K, D]   (B=512, K=512, D=1024)
    mask = round(clip(M, 0, 1)) = (M > 0.5)
    D_out[b, k] = sum_d mask[k,d] * (X[b,d] - C[k,d])^2

Algebraic expansion (mask^2 = mask):
    D_out[b,k] = sum_d mask*X^2  - 2*sum_d (mask*C)*X  + sum_d mask*C^2

Sharding: each of the 8 cores owns a 64-row shard of C / mask / D_out^T
(out_dim shard) -> every core needs full X but no collectives at all.

Per-core dataflow (everything d-major "transposed" layout, d on partitions):
    C^raw = U_s^T X            (PE, bf16, PSUM [2x64, 512])
    Ĉ = -2/B * C^raw           (ACT scale-copy)          [128, 512] f32
    Ĉᵀ chunks via PE transpose -> PSUM  (so d is on partitions)
    maskᵀ = (Mᵀ > 0.5)         (PE transpose of fp32 M, then DVE is_gt -> bf16)
    CMᵀ  = maskᵀ * Ĉᵀ = -2*mask*C      (DVE -> bf16)
    Gᵀ   = CMᵀ * Ĉᵀ  = 4*mask*C^2      (DVE -> bf16)
    X2ᵀ  = XTᵀ * XTᵀ                    (DVE/ACT -> bf16)
    Dᵀ  += maskᵀ.T @ X2ᵀ  (T1)          (PE accum in PSUM [64, 512])
    Dᵀ  += CMᵀ.T  @ XTᵀ   (-2*T2)
    t3   = colsum(Gᵀ)/4   (PE ones-matmuls -> [64,1], ACT scale 0.25)
    Dᵀ_s = Dᵀ + t3 (per-partition scalar add, DVE) -> DMA out [64, 512] f32

Host: casts X to bf16 in both layouts (layout/dtype prep only; all FLOPs of
the algorithm run on device), shards U/M, gathers Dᵀ shards.
"""

import numpy as np

BATCH = 512
OUT_DIM = 512
IN_DIM = 1024
N_CORES = 8
KS = OUT_DIM // N_CORES  # 64 centroid rows per core

_CACHE = {}


def build_module(num_devices: int = N_CORES):
    """Build + compile the Bass module (same SPMD program for every core)."""
    import concourse.bacc as bacc
    import concourse.mybir as mybir
    from concourse import tile
    from concourse.masks import make_identity

    if num_devices in _CACHE:
        return _CACHE[num_devices]

    fp32 = mybir.dt.float32
    bf16 = mybir.dt.bfloat16
    Alu = mybir.AluOpType
    Act = mybir.ActivationFunctionType

    nc = bacc.Bacc("TRN2", target_bir_lowering=False, debug=False,
                   num_devices=num_devices)

    xb = nc.dram_tensor("xb", [BATCH, IN_DIM], bf16, kind="ExternalInput").ap()
    xt = nc.dram_tensor("xt", [IN_DIM, BATCH], bf16, kind="ExternalInput").ap()
    us = nc.dram_tensor("us", [BATCH, KS], bf16, kind="ExternalInput").ap()
    ms = nc.dram_tensor("ms", [KS, IN_DIM], fp32, kind="ExternalInput").ap()
    dt_out = nc.dram_tensor("dt", [KS, BATCH], fp32, kind="ExternalOutput").ap()

    NB = BATCH // 128   # 4 b-chunks
    ND = IN_DIM // 128  # 8 d-chunks

    with tile.TileContext(nc) as tc:
        with (
            tc.tile_pool(name="const", bufs=1) as constp,
            tc.tile_pool(name="xbp", bufs=1) as xbp,
            tc.tile_pool(name="xtp", bufs=1) as xtp,
            tc.tile_pool(name="x2tp", bufs=1) as x2tp,
            tc.tile_pool(name="smal", bufs=1) as smal,
            tc.tile_pool(name="psum", bufs=1, space="PSUM") as psp,
        ):
            # ---- constants
            ident = constp.tile([64, 64], fp32, tag="ident")
            make_identity(nc, ident[:, :])
            ones_col = constp.tile([128, 1], bf16, tag="ones")
            nc.vector.memset(ones_col[:, :], 1.0)

            # ---- DMA in
            xb_t = [xbp.tile([128, IN_DIM], bf16, tag=f"xb{i}", name=f"xb{i}") for i in range(NB)]
            for i in range(NB):
                nc.sync.dma_start(xb_t[i][:, :], xb[128 * i:128 * (i + 1), :])

            us_sb = smal.tile([128, NB * KS], bf16, tag="us")
            nc.sync.dma_start(
                us_sb[:, :].rearrange("p (i k) -> p i k", i=NB),
                us.rearrange("(i p) k -> p i k", p=128),
            )

            ms_sb = smal.tile([KS, IN_DIM], fp32, tag="ms")
            nc.sync.dma_start(ms_sb[:, :], ms[:, :])

            xt_t = [xtp.tile([128, BATCH], bf16, tag=f"xt{j}", name=f"xt{j}") for j in range(ND)]
            for j in range(ND):
                nc.sync.dma_start(xt_t[j][:, :], xt[128 * j:128 * (j + 1), :])

            # ---- C^raw = U_s^T X  (two d-halves, separate PSUM banks so all
            # matmul operands stay at base partition 0)
            psum_ch = [psp.tile([64, 512], fp32, tag=f"pc{h}", name=f"pc{h}")
                       for h in range(2)]
            for h in range(2):
                for i in range(NB):
                    nc.tensor.matmul(
                        psum_ch[h][:, :],
                        us_sb[:, KS * i:KS * (i + 1)],
                        xb_t[i][:, 512 * h:512 * (h + 1)],
                        start=(i == 0), stop=(i == NB - 1),
                    )
            # Ĉ = -2/B * C^raw   (so later products fold the -2 T2 factor)
            c_sb = smal.tile([64, IN_DIM], fp32, tag="c")
            for h in range(2):
                nc.scalar.activation(c_sb[:, 512 * h:512 * (h + 1)],
                                     psum_ch[h][:, :], Act.Copy,
                                     scale=-2.0 / BATCH)

            # ---- transpose Ĉ -> Ĉᵀ packed [128(d), 8*64(k)] in PSUM
            psum_ct = psp.tile([128, 512], fp32, tag="pct")
            for j in range(ND):
                nc.tensor.transpose(
                    psum_ct[:, 64 * j:64 * (j + 1)],
                    c_sb[:, 128 * j:128 * (j + 1)],
                    ident[:, :],
                )

            # ---- maskᵀ: transpose fp32 M, then threshold > 0.5 -> bf16
            psum_mt = psp.tile([128, 512], fp32, tag="pmt")
            for j in range(ND):
                nc.tensor.transpose(
                    psum_mt[:, 64 * j:64 * (j + 1)],
                    ms_sb[:, 128 * j:128 * (j + 1)],
                    ident[0:64, :],
                )
            maskt = smal.tile([128, 512], bf16, tag="maskt")
            nc.vector.tensor_scalar(maskt[:, :], psum_mt[:, :], 0.5, None,
                                    Alu.is_gt)

            # ---- CMᵀ = maskᵀ * Ĉᵀ (= -2 mask C),  Gᵀ = CMᵀ * Ĉᵀ (= 4 mask C²)
            cmt = smal.tile([128, 512], bf16, tag="cmt")
            nc.vector.tensor_tensor(cmt[:, :], maskt[:, :], psum_ct[:, :],
                                    Alu.mult)
            g_sb = smal.tile([128, 512], bf16, tag="g")
            nc.vector.tensor_tensor(g_sb[:, :], cmt[:, :], psum_ct[:, :],
                                    Alu.mult)

            # ---- X2ᵀ = XTᵀ²  (split DVE / ACT)
            x2t_t = [x2tp.tile([128, BATCH], bf16, tag=f"x2t{j}", name=f"x2t{j}") for j in range(ND)]
            for j in range(ND):
                if j % 2 == 0:
                    nc.vector.tensor_tensor(x2t_t[j][:, :], xt_t[j][:, :],
                                            xt_t[j][:, :], Alu.mult)
                else:
                    nc.scalar.activation(x2t_t[j][:, :], xt_t[j][:, :],
                                         Act.Square)

            # ---- Dᵀ accumulation: T1 then -2*T2
            psum_d = psp.tile([64, 512], fp32, tag="pd")
            for j in range(ND):
                nc.tensor.matmul(psum_d[:, :], maskt[:, 64 * j:64 * (j + 1)],
                                 x2t_t[j][:, :],
                                 start=(j == 0), stop=False)
            for j in range(ND):
                nc.tensor.matmul(psum_d[:, :], cmt[:, 64 * j:64 * (j + 1)],
                                 xt_t[j][:, :],
                                 start=False, stop=(j == ND - 1))

            # ---- t3 = sum_d mask*C²: colsum of Gᵀ chunks -> [64, 1]
            psum_t3 = psp.tile([64, 1], fp32, tag="pt3")
            for j in range(ND):
                nc.tensor.matmul(psum_t3[:, :], g_sb[:, 64 * j:64 * (j + 1)],
                                 ones_col[:, :],
                                 start=(j == 0), stop=(j == ND - 1))
            t3s = smal.tile([64, 1], fp32, tag="t3s")
            nc.scalar.activation(t3s[:, :], psum_t3[:, :], Act.Copy, scale=0.25)

            # ---- Dᵀ_s = Dᵀ + t3 (per-partition scalar) -> DRAM
            d_sb = smal.tile([64, 512], fp32, tag="d")
            nc.vector.tensor_scalar(d_sb[:, :], psum_d[:, :], t3s[:, 0:1], None,
                                    Alu.add)
            nc.sync.dma_start(dt_out[:, :], d_sb[:, :])

    nc.compile()
    _CACHE[num_devices] = nc
    return nc


def kernel(X: np.ndarray, U: np.ndarray, M: np.ndarray) -> np.ndarray:
    import ml_dtypes
    from concourse import bass_utils

    nc = build_module(N_CORES)

    bf16 = ml_dtypes.bfloat16
    xb_np = np.ascontiguousarray(X).astype(bf16)
    xt_np = np.ascontiguousarray(X.T).astype(bf16)

    in_maps = []
    for c in range(N_CORES):
        in_maps.append({
            "xb": xb_np,
            "xt": xt_np,
            "us": np.ascontiguousarray(U[:, KS * c:KS * (c + 1)]).astype(bf16),
            "ms": np.ascontiguousarray(M[KS * c:KS * (c + 1), :]).astype(np.float32),
        })

    res = bass_utils.run_bass_kernel_spmd(nc, in_maps, core_ids=list(range(N_CORES)))

    out = np.empty((BATCH, OUT_DIM), dtype=np.float32)
    for c in range(N_CORES):
        out[:, KS * c:KS * (c + 1)] = res.results[c]["dt"].T
    return out


# revision 9
# speedup vs baseline: 1.1794x; 1.1794x over previous
"""TRN2 Bass kernel: masked-centroid squared distances (8 NeuronCores, SPMD).

Reference computation (fp32):
    C = U^T X / B                          [K, D]   (B=512, K=512, D=1024)
    mask = round(clip(M, 0, 1)) = (M > 0.5)
    D_out[b, k] = sum_d mask[k,d] * (X[b,d] - C[k,d])^2

Algebraic expansion (mask^2 = mask):
    D_out[b,k] = sum_d mask*X^2  - 2*sum_d (mask*C)*X  + sum_d mask*C^2

Sharding: each of the 8 cores owns a 64-row shard of C / mask / D_out^T
(out_dim shard) -> every core needs full X but no collectives at all.

Per-core dataflow (d-major "transposed" layout, d on partitions, bf16 MMs):
    C^raw = U_s^T X            (PE, bf16, 2 PSUM banks [64, 512])
    Ĉ = -2/B * C^raw           (ACT scale-copy -> [64, 1024] f32)
    Ĉᵀ chunks via PE transpose -> PSUM [128, 8*64]
    maskᵀ = (Mᵀ > 0.5)         (PE transpose of fp32 M, then DVE is_gt -> bf16)
    CMᵀ  = maskᵀ * Ĉᵀ = -2*mask*C      (DVE -> bf16)
    Gᵀ   = CMᵀ * Ĉᵀ  = 4*mask*C^2      (DVE -> bf16)
    X2ᵀ  = XTᵀ * XTᵀ                    (DVE/ACT -> bf16)
    Dᵀ  += maskᵀ.T @ X2ᵀ  (T1)          (PE accum in PSUM [64, 512])
    Dᵀ  += CMᵀ.T  @ XTᵀ   (-2*T2)
    t3   = colsum(Gᵀ) -> [64,1] (PE), t3f = t3/4 -> [1,64]-like scalar lane
    Dᵀ  += t3f ⊗ 1  (rank-1 K=1 matmul folded into the same PSUM group)
    Dᵀ -> SBUF (split DVE/ACT copies) -> DMA out [64, 512] f32

The PE p-state ramps to full clock only after ~3 µs of *continuous* busy
(HAM clock gate); warm-up and filler matmuls keep the PE dense so the big
matmuls run at full rate.

Host: casts X to bf16 in both layouts (layout/dtype prep only; all FLOPs of
the algorithm run on device), shards U/M, gathers Dᵀ shards.
"""

import numpy as np

BATCH = 512
OUT_DIM = 512
IN_DIM = 1024
N_CORES = 8
KS = OUT_DIM // N_CORES  # 64 centroid rows per core

_CACHE = {}


def build_module(num_devices: int = N_CORES):
    """Build + compile the Bass module (same SPMD program for every core)."""
    import concourse.bacc as bacc
    import concourse.mybir as mybir
    from concourse import tile
    from concourse.masks import make_identity

    if num_devices in _CACHE:
        return _CACHE[num_devices]

    fp32 = mybir.dt.float32
    bf16 = mybir.dt.bfloat16
    Alu = mybir.AluOpType
    Act = mybir.ActivationFunctionType

    nc = bacc.Bacc("TRN2", target_bir_lowering=False, debug=False,
                   num_devices=num_devices)

    xb = nc.dram_tensor("xb", [BATCH, IN_DIM], bf16, kind="ExternalInput").ap()
    xt = nc.dram_tensor("xt", [IN_DIM, BATCH], bf16, kind="ExternalInput").ap()
    us = nc.dram_tensor("us", [BATCH, KS], bf16, kind="ExternalInput").ap()
    ms = nc.dram_tensor("ms", [KS, IN_DIM], fp32, kind="ExternalInput").ap()
    dt_out = nc.dram_tensor("dt", [KS, BATCH], fp32, kind="ExternalOutput").ap()

    NB = BATCH // 128   # 4 b-chunks
    ND = IN_DIM // 128  # 8 d-chunks

    with tile.TileContext(nc) as tc:
        with (
            tc.tile_pool(name="const", bufs=1) as constp,
            tc.tile_pool(name="xbp", bufs=1) as xbp,
            tc.tile_pool(name="xtp", bufs=1) as xtp,
            tc.tile_pool(name="x2tp", bufs=1) as x2tp,
            tc.tile_pool(name="smal", bufs=1) as smal,
            tc.tile_pool(name="psum", bufs=1, space="PSUM") as psp,
        ):
            # ---- constants
            ident = constp.tile([64, 64], fp32, tag="ident")
            make_identity(nc, ident[:, :])
            wtile = constp.tile([128, 512], bf16, tag="wtile")
            nc.gpsimd.memset(wtile[:, :], 0.0)

            # ---- DMA in (order = arrival order; C-chain inputs first, the
            # T1/T2 moving operand (xt) last, with a small final chunk so the
            # post-DMA tail is short)
            us_sb = smal.tile([128, NB * KS], bf16, tag="us")
            nc.sync.dma_start(
                us_sb[:, :].rearrange("p (i k) -> p i k", i=NB),
                us.rearrange("(i p) k -> p i k", p=128),
            )
            # xb in two [128, 2048] halves (b-chunks {0,1} and {2,3})
            xb_t = [xbp.tile([128, 2 * IN_DIM], bf16, tag=f"xb{a}", name=f"xb{a}")
                    for a in range(2)]
            for a in range(2):
                nc.sync.dma_start(
                    xb_t[a][:, :].rearrange("p (i d) -> p i d", i=2),
                    xb[256 * a:256 * (a + 1), :].rearrange("(i p) d -> p i d", p=128),
                )

            ms_sb = smal.tile([KS, IN_DIM], fp32, tag="ms")
            nc.sync.dma_start(ms_sb[:, :], ms[:, :])

            # xt: three [128, 1024] chunks (d-chunks {0..5}) + two small
            # [128, 512] chunks (d-chunks 6, 7) landing last
            xt_q = [xtp.tile([128, 2 * BATCH], bf16, tag=f"xtq{q}", name=f"xtq{q}")
                    for q in range(3)]
            for q in range(3):
                nc.sync.dma_start(
                    xt_q[q][:, :].rearrange("p (r b) -> p r b", r=2),
                    xt[256 * q:256 * (q + 1), :].rearrange("(r p) b -> p r b", p=128),
                )
            xt_s = [xtp.tile([128, BATCH], bf16, tag=f"xts{j}", name=f"xts{j}")
                    for j in (6, 7)]
            for idx, j in enumerate((6, 7)):
                nc.sync.dma_start(xt_s[idx][:, :], xt[128 * j:128 * (j + 1), :])

            def xt_slice(j):
                if j < 6:
                    return xt_q[j // 2][:, 512 * (j % 2):512 * (j % 2 + 1)]
                return xt_s[j - 6][:, :]

            # ---- PE warm-up: dummy matmuls with no data deps keep the PE
            # clock ramping while DMAs land.  They write psum_d, which the
            # first real T1 matmul later resets with start=True.
            psum_d = psp.tile([64, 512], fp32, tag="pd")

            def dummy_mm(n=512):
                nc.tensor.matmul(psum_d[:, 0:n], wtile[:, 0:64], wtile[:, 0:n],
                                 start=True, stop=True)

            for _ in range(3):
                dummy_mm()

            # ---- C^raw = U_s^T X   (two d-half groups; i-major so each xb
            # half is consumed on arrival)
            psum_ch = [psp.tile([64, 512], fp32, tag=f"pc{h}", name=f"pc{h}")
                       for h in range(2)]
            for i in range(NB):
                for h in range(2):
                    nc.tensor.matmul(
                        psum_ch[h][:, :],
                        us_sb[:, KS * i:KS * (i + 1)],
                        xb_t[i // 2][:, IN_DIM * (i % 2) + 512 * h:
                                     IN_DIM * (i % 2) + 512 * (h + 1)],
                        start=(i == 0), stop=(i == NB - 1),
                    )
                if i == 1:
                    dummy_mm()  # bridge the wait for the second xb half

            # ---- maskᵀ: transpose fp32 M, then threshold > 0.5 -> bf16
            psum_mt = psp.tile([128, 512], fp32, tag="pmt")
            for j in range(ND):
                nc.tensor.transpose(
                    psum_mt[:, 64 * j:64 * (j + 1)],
                    ms_sb[:, 128 * j:128 * (j + 1)],
                    ident[:, :],
                )
            maskt = smal.tile([128, 512], bf16, tag="maskt")
            nc.vector.tensor_scalar(maskt[:, :], psum_mt[:, :], 0.5, None,
                                    Alu.is_gt)

            # Ĉ = -2/B * C^raw
            c_sb = smal.tile([64, IN_DIM], fp32, tag="c")
            for h in range(2):
                nc.scalar.activation(c_sb[:, 512 * h:512 * (h + 1)],
                                     psum_ch[h][:, :], Act.Copy,
                                     scale=-2.0 / BATCH)

            dummy_mm()  # keep PE busy while ACT copies Ĉ

            # ---- transpose Ĉ -> Ĉᵀ packed [128(d), 8*64(k)] in PSUM
            psum_ct = psp.tile([128, 512], fp32, tag="pct")
            for j in range(ND):
                nc.tensor.transpose(
                    psum_ct[:, 64 * j:64 * (j + 1)],
                    c_sb[:, 128 * j:128 * (j + 1)],
                    ident[:, :],
                )

            # ---- CMᵀ = maskᵀ * Ĉᵀ,  Gᵀ = CMᵀ * Ĉᵀ  (DVE; g only feeds t3)
            cmt = smal.tile([128, 512], bf16, tag="cmt")
            nc.vector.tensor_tensor(cmt[:, :], maskt[:, :], psum_ct[:, :],
                                    Alu.mult)

            # ---- X2ᵀ squares: q0 ACT, q1 DVE, q2 DVE, j6 ACT, j7 DVE
            x2t_q = [x2tp.tile([128, 2 * BATCH], bf16, tag=f"x2q{q}", name=f"x2q{q}")
                     for q in range(3)]
            x2t_s = [x2tp.tile([128, BATCH], bf16, tag=f"x2s{j}", name=f"x2s{j}")
                     for j in (6, 7)]

            def x2t_slice(j):
                if j < 6:
                    return x2t_q[j // 2][:, 512 * (j % 2):512 * (j % 2 + 1)]
                return x2t_s[j - 6][:, :]

            nc.scalar.activation(x2t_q[0][:, :], xt_q[0][:, :], Act.Square)
            nc.vector.tensor_tensor(x2t_q[1][:, :], xt_q[1][:, :],
                                    xt_q[1][:, :], Alu.mult)
            nc.vector.tensor_tensor(x2t_q[2][:, :], xt_q[2][:, :],
                                    xt_q[2][:, :], Alu.mult)
            g_sb = smal.tile([128, 512], bf16, tag="g")
            nc.vector.tensor_tensor(g_sb[:, :], cmt[:, :], psum_ct[:, :],
                                    Alu.mult)
            nc.scalar.activation(x2t_s[0][:, :], xt_s[0][:, :], Act.Square)
            nc.vector.tensor_tensor(x2t_s[1][:, :], xt_s[1][:, :],
                                    xt_s[1][:, :], Alu.mult)

            # ---- Dᵀ accumulation: T1/T2 for d-chunks 0..5, then t3, then
            # the late chunks 6..7, then the rank-1 t3 add closes the group.
            for j in range(6):
                nc.tensor.matmul(psum_d[:, :], maskt[:, 64 * j:64 * (j + 1)],
                                 x2t_slice(j), start=(j == 0), stop=False)
            for j in range(6):
                nc.tensor.matmul(psum_d[:, :], cmt[:, 64 * j:64 * (j + 1)],
                                 xt_slice(j), start=False, stop=False)

            # t3 colsums (ready as soon as g is; fills the wait for xt6/7)
            ones_col = constp.tile([128, 1], bf16, tag="ones")
            nc.vector.memset(ones_col[:, :], 1.0)
            psum_t3 = psp.tile([1, 64], fp32, tag="pt3")
            for j in range(ND):
                nc.tensor.matmul(psum_t3[:, :], ones_col[:, :],
                                 g_sb[:, 64 * j:64 * (j + 1)],
                                 start=(j == 0), stop=(j == ND - 1))
            t3f = smal.tile([1, 64], bf16, tag="t3f")
            nc.scalar.activation(t3f[:, :], psum_t3[:, :], Act.Copy, scale=0.25)

            for j in (6, 7):
                nc.tensor.matmul(psum_d[:, :], maskt[:, 64 * j:64 * (j + 1)],
                                 x2t_slice(j), start=False, stop=False)
                nc.tensor.matmul(psum_d[:, :], cmt[:, 64 * j:64 * (j + 1)],
                                 xt_slice(j), start=False, stop=False)

            onesrow = constp.tile([1, 512], bf16, tag="onesrow")
            nc.vector.memset(onesrow[:, :], 1.0)
            nc.tensor.matmul(psum_d[:, :], t3f[:, :], onesrow[:, :],
                             start=False, stop=True)

            # ---- Dᵀ -> SBUF (two parallel half-copies) -> DRAM
            d_sb = smal.tile([64, 512], fp32, tag="d")
            nc.vector.tensor_copy(d_sb[:, 0:256], psum_d[:, 0:256])
            nc.scalar.copy(d_sb[:, 256:512], psum_d[:, 256:512])
            nc.sync.dma_start(dt_out[:, :], d_sb[:, :])

    nc.compile()
    _CACHE[num_devices] = nc
    return nc


def kernel(X: np.ndarray, U: np.ndarray, M: np.ndarray) -> np.ndarray:
    import ml_dtypes
    from concourse import bass_utils

    nc = build_module(N_CORES)

    bf16 = ml_dtypes.bfloat16
    xb_np = np.ascontiguousarray(X).astype(bf16)
    xt_np = np.ascontiguousarray(X.T).astype(bf16)

    in_maps = []
    for c in range(N_CORES):
        in_maps.append({
            "xb": xb_np,
            "xt": xt_np,
            "us": np.ascontiguousarray(U[:, KS * c:KS * (c + 1)]).astype(bf16),
            "ms": np.ascontiguousarray(M[KS * c:KS * (c + 1), :]).astype(np.float32),
        })

    res = bass_utils.run_bass_kernel_spmd(nc, in_maps, core_ids=list(range(N_CORES)))

    out = np.empty((BATCH, OUT_DIM), dtype=np.float32)
    for c in range(N_CORES):
        out[:, KS * c:KS * (c + 1)] = res.results[c]["dt"].T
    return out


# revision 16
# speedup vs baseline: 1.3334x; 1.1306x over previous
"""TRN2 Bass kernel: masked-centroid squared distances (8 NeuronCores, SPMD).

Reference computation (fp32):
    C = U^T X / B                          [K, D]   (B=512, K=512, D=1024)
    mask = round(clip(M, 0, 1)) = (M > 0.5)
    D_out[b, k] = sum_d mask[k,d] * (X[b,d] - C[k,d])^2

Algebraic expansion (mask^2 = mask):
    D_out[b,k] = sum_d mask*X^2  - 2*sum_d (mask*C)*X  + sum_d mask*C^2

Sharding: each of the 8 cores owns a 64-row shard of C / mask / D_out^T
(out_dim shard) -> every core needs full X but no collectives at all.

Per-core dataflow (d-major layout, d on partitions for the big matmuls):
    Ĉᵀraw[d,k] chunks = sum_b X[b,d] U_s[b,k]   (PE, fp8 ops, direct into the
        transposed layout: lhsT = X b-chunk, rhs = U_s b-chunk -> [128, 64])
    maskᵀ = (Mᵀ > 0.5)            (PE transpose of fp32 M, DVE is_gt -> bf16)
    CMᵀ  = (Ĉᵀraw * -1/256) * maskᵀ = -2*mask*C     (fused DVE stt -> bf16)
    Gᵀ   = (Ĉᵀraw * -1/256) * CMᵀ  = 4*mask*C^2    (fused DVE stt -> bf16)
    X2ᵀ  = XTᵀ * XTᵀ                                (DVE/ACT -> bf16)
    Dᵀ  += maskᵀ.T @ X2ᵀ   (T1)     (PE bf16, accum in PSUM [64, 512])
    Dᵀ  += CMᵀ.T  @ XTᵀ    (-2*T2)
    t3   = colsum(Gᵀ)/4 -> [1, 64]  (PE ones-colsum + ACT scale)
    Dᵀ  += t3 ⊗ 1   (rank-1 K=1 matmul closes the same PSUM group)
    Dᵀ -> SBUF (split DVE/ACT half-copies) -> DMA out [64, 512] f32

Precision: X enters the distance terms in bf16 (both layouts).  X and U enter
the *centroid* matmul in fp8e4m3 — C is ~40x smaller than X in magnitude and
only enters D through second-order terms, so fp8's ~4% element error adds
~1e-4 relative error to D while cutting the centroid operands' DMA 4x.
M stays fp32: the mask threshold (M > 0.5) must match the fp32 reference
bit-for-bit near 0.5.

The PE p-state ramps to full clock only after sustained busy (HAM clock
gate); warm-up/filler matmuls keep the PE dense through the real work.

Host does layout/dtype prep only (casts, transpose, shard, gather); all
FLOPs of the algorithm run on device.
"""

import numpy as np

BATCH = 512
OUT_DIM = 512
IN_DIM = 1024
N_CORES = 8
KS = OUT_DIM // N_CORES  # 64 centroid rows per core

_CACHE = {}


def build_module(num_devices: int = N_CORES):
    """Build + compile the Bass module (same SPMD program for every core)."""
    import concourse.bacc as bacc
    import concourse.mybir as mybir
    from concourse import tile
    from concourse.masks import make_identity

    if num_devices in _CACHE:
        return _CACHE[num_devices]

    fp32 = mybir.dt.float32
    bf16 = mybir.dt.bfloat16
    fp8 = mybir.dt.float8e4
    Alu = mybir.AluOpType
    Act = mybir.ActivationFunctionType

    nc = bacc.Bacc("TRN2", target_bir_lowering=False, debug=False,
                   num_devices=num_devices)

    NB = BATCH // 128   # 4 b-chunks
    ND = IN_DIM // 128  # 8 d-chunks

    # xb arrives d-chunk-major: xbj[m][p, 256*i + dd] = X[128*i + p, 256*m + dd]
    # so the j-major centroid accumulation groups pace with the DMA stream.
    xb = nc.dram_tensor("xb", [NB, 128, IN_DIM], fp8, kind="ExternalInput").ap()
    xt = nc.dram_tensor("xt", [IN_DIM, BATCH], bf16, kind="ExternalInput").ap()
    us = nc.dram_tensor("us", [BATCH, KS], fp8, kind="ExternalInput").ap()
    ms = nc.dram_tensor("ms", [KS, IN_DIM], fp32, kind="ExternalInput").ap()
    dt_out = nc.dram_tensor("dt", [KS, BATCH], fp32, kind="ExternalOutput").ap()

    with tile.TileContext(nc) as tc:
        with (
            tc.tile_pool(name="const", bufs=1) as constp,
            tc.tile_pool(name="xbp", bufs=1) as xbp,
            tc.tile_pool(name="xtp", bufs=1) as xtp,
            tc.tile_pool(name="x2tp", bufs=1) as x2tp,
            tc.tile_pool(name="smal", bufs=1) as smal,
            tc.tile_pool(name="psum", bufs=1, space="PSUM") as psp,
        ):
            # ---- constants
            ident = constp.tile([64, 64], fp32, tag="ident")
            make_identity(nc, ident[:, :])
            wtile = constp.tile([128, 512], bf16, tag="wtile")
            nc.gpsimd.memset(wtile[:, :], 0.0)
            ones_col = constp.tile([128, 1], bf16, tag="ones")
            nc.vector.memset(ones_col[:, :], 1.0)
            onesrow = constp.tile([1, 512], bf16, tag="onesrow")
            nc.vector.memset(onesrow[:, :], 1.0)

            # ---- DMA in.  Arrival order: mask source first (its chain is
            # engine-latency-bound), centroid operands next, the T1/T2 moving
            # operand (xt) last with small final chunks for a short tail.
            ms_sb = smal.tile([KS, IN_DIM], fp32, tag="ms")
            nc.sync.dma_start(ms_sb[:, :], ms[:, :])

            us_sb = smal.tile([128, NB * KS], fp8, tag="us")
            nc.sync.dma_start(
                us_sb[:, :].rearrange("p (i k) -> p i k", i=NB),
                us.rearrange("(i p) k -> p i k", p=128),
            )
            # xb in two fp8 halves [128, 2048], d-chunk-major: half a holds
            # d-chunks {4a..4a+3}; layout [p, 1024*m' + 256*i + dd]
            xb_t = [xbp.tile([128, 2 * IN_DIM], fp8, tag=f"xb{a}", name=f"xb{a}")
                    for a in range(2)]
            for a in range(2):
                nc.sync.dma_start(
                    xb_t[a][:, :].rearrange("p (m v) -> p m v", m=2),
                    xb[2 * a:2 * (a + 1), :, :].rearrange("m p v -> p m v"),
                )

            # xt: three [128, 1024] chunks (d-chunks {0..5}) + two [128, 512]
            xt_q = [xtp.tile([128, 2 * BATCH], bf16, tag=f"xtq{q}", name=f"xtq{q}")
                    for q in range(3)]
            for q in range(3):
                nc.sync.dma_start(
                    xt_q[q][:, :].rearrange("p (r b) -> p r b", r=2),
                    xt[256 * q:256 * (q + 1), :].rearrange("(r p) b -> p r b", p=128),
                )
            xt_s = [xtp.tile([128, BATCH], bf16, tag=f"xts{j}", name=f"xts{j}")
                    for j in (6, 7)]
            for idx, j in enumerate((6, 7)):
                nc.sync.dma_start(xt_s[idx][:, :], xt[128 * j:128 * (j + 1), :])

            def xt_slice(j):
                if j < 6:
                    return xt_q[j // 2][:, 512 * (j % 2):512 * (j % 2 + 1)]
                return xt_s[j - 6][:, :]

            # ---- PE warm-up: dummy matmuls (no data deps) ramp the PE clock
            # while DMAs land; they write psum_d which T1-j0 later resets.
            psum_d = psp.tile([64, 512], fp32, tag="pd")

            def dummy_mm(n=512):
                nc.tensor.matmul(psum_d[:, 0:n], wtile[:, 0:64], wtile[:, 0:n],
                                 start=True, stop=True)

            for _ in range(3):
                dummy_mm()

            # ---- maskᵀ: transpose fp32 M, then threshold > 0.5 -> bf16
            psum_mt = psp.tile([128, 512], fp32, tag="pmt")
            for j in range(ND):
                nc.tensor.transpose(
                    psum_mt[:, 64 * j:64 * (j + 1)],
                    ms_sb[:, 128 * j:128 * (j + 1)],
                    ident[:, :],
                )
            maskt = smal.tile([128, 512], bf16, tag="maskt")
            nc.vector.tensor_scalar(maskt[:, :], psum_mt[:, :], 0.5, None,
                                    Alu.is_gt)

            # ---- Ĉᵀraw[d,k] direct: per d-chunk j accumulate over b-chunks.
            # lhsT = X[b-chunk, d-chunk] (fp8), rhs = U_s[b-chunk] (fp8).
            # j-major (one pending PSUM accumulation group at a time); each
            # xb half covers 4 whole j-groups, so pacing is preserved.
            psum_ct = psp.tile([128, 512], fp32, tag="pct")
            for j in range(ND):
                a, mm = divmod(j, 4)  # xb half a, chunk-pair m'=mm//2, r=mm%2
                base = 1024 * (mm // 2) + 128 * (mm % 2)
                for i in range(NB):
                    nc.tensor.matmul(
                        psum_ct[:, 64 * j:64 * (j + 1)],
                        xb_t[a][:, base + 256 * i:base + 256 * i + 128],
                        us_sb[:, KS * i:KS * (i + 1)],
                        start=(i == 0), stop=(i == NB - 1),
                    )
                if j == 3:
                    dummy_mm()  # bridge the wait for the second xb half

            # ---- X2ᵀ squares + fused CM/G products (DVE/ACT split)
            x2t_q = [x2tp.tile([128, 2 * BATCH], bf16, tag=f"x2q{q}", name=f"x2q{q}")
                     for q in range(3)]
            x2t_s = [x2tp.tile([128, BATCH], bf16, tag=f"x2s{j}", name=f"x2s{j}")
                     for j in (6, 7)]

            def x2t_slice(j):
                if j < 6:
                    return x2t_q[j // 2][:, 512 * (j % 2):512 * (j % 2 + 1)]
                return x2t_s[j - 6][:, :]

            # DVE: sq0, cmt, sq2, sq7, g   /  ACT: sq1, sq6, t3 scale
            nc.vector.tensor_tensor(x2t_q[0][:, :], xt_q[0][:, :],
                                    xt_q[0][:, :], Alu.mult)
            cmt = smal.tile([128, 512], bf16, tag="cmt")
            nc.vector.scalar_tensor_tensor(cmt[:, :], psum_ct[:, :], -1.0 / 256.0,
                                           maskt[:, :], Alu.mult, Alu.mult)
            nc.scalar.activation(x2t_q[1][:, :], xt_q[1][:, :], Act.Square)
            nc.vector.tensor_tensor(x2t_q[2][:, :], xt_q[2][:, :],
                                    xt_q[2][:, :], Alu.mult)
            nc.scalar.activation(x2t_s[0][:, :], xt_s[0][:, :], Act.Square)
            nc.vector.tensor_tensor(x2t_s[1][:, :], xt_s[1][:, :],
                                    xt_s[1][:, :], Alu.mult)
            g_sb = smal.tile([128, 512], bf16, tag="g")
            nc.vector.scalar_tensor_tensor(g_sb[:, :], psum_ct[:, :], -1.0 / 256.0,
                                           cmt[:, :], Alu.mult, Alu.mult)

            # ---- Dᵀ accumulation, interleaved to match operand arrival
            def t1(j, start=False):
                nc.tensor.matmul(psum_d[:, :], maskt[:, 64 * j:64 * (j + 1)],
                                 x2t_slice(j), start=start, stop=False)

            def t2(j):
                nc.tensor.matmul(psum_d[:, :], cmt[:, 64 * j:64 * (j + 1)],
                                 xt_slice(j), start=False, stop=False)

            t1(0, start=True)
            t1(1)
            t2(0)
            t2(1)
            t2(2)
            t1(2)
            t1(3)
            t2(3)
            t2(4)
            t1(4)
            t1(5)
            t2(5)

            # t3 colsums (ready with g; fill the wait for xt6/7)
            psum_t3 = psp.tile([1, 64], fp32, tag="pt3")
            for j in range(ND):
                nc.tensor.matmul(psum_t3[:, :], ones_col[:, :],
                                 g_sb[:, 64 * j:64 * (j + 1)],
                                 start=(j == 0), stop=(j == ND - 1))
            t3f = smal.tile([1, 64], bf16, tag="t3f")
            nc.scalar.activation(t3f[:, :], psum_t3[:, :], Act.Copy, scale=0.25)

            t1(6)
            t2(6)
            t1(7)
            t2(7)
            nc.tensor.matmul(psum_d[:, :], t3f[:, :], onesrow[:, :],
                             start=False, stop=True)

            # ---- Dᵀ -> SBUF (two parallel half-copies) -> DRAM
            d_sb = smal.tile([64, 512], fp32, tag="d")
            nc.vector.tensor_copy(d_sb[:, 0:256], psum_d[:, 0:256])
            nc.scalar.copy(d_sb[:, 256:512], psum_d[:, 256:512])
            nc.sync.dma_start(dt_out[:, :], d_sb[:, :])

    nc.compile()
    _CACHE[num_devices] = nc
    return nc


def kernel(X: np.ndarray, U: np.ndarray, M: np.ndarray) -> np.ndarray:
    import ml_dtypes
    from concourse import bass_utils

    nc = build_module(N_CORES)

    bf16 = ml_dtypes.bfloat16
    fp8 = ml_dtypes.float8_e4m3
    # d-chunk-major fp8 layout: xbj[m][p, 256*i + dd] = X[128*i + p, 256*m + dd]
    xb_np = np.ascontiguousarray(
        X.reshape(4, 128, 4, 256).transpose(2, 1, 0, 3).reshape(4, 128, 1024)
    ).astype(fp8)
    xt_np = np.ascontiguousarray(X.T).astype(bf16)

    in_maps = []
    for c in range(N_CORES):
        in_maps.append({
            "xb": xb_np,
            "xt": xt_np,
            "us": np.ascontiguousarray(U[:, KS * c:KS * (c + 1)]).astype(fp8),
            "ms": np.ascontiguousarray(M[KS * c:KS * (c + 1), :]).astype(np.float32),
        })

    res = bass_utils.run_bass_kernel_spmd(nc, in_maps, core_ids=list(range(N_CORES)))

    out = np.empty((BATCH, OUT_DIM), dtype=np.float32)
    for c in range(N_CORES):
        out[:, KS * c:KS * (c + 1)] = res.results[c]["dt"].T
    return out


# revision 19
# speedup vs baseline: 1.4225x; 1.0668x over previous
"""TRN2 Bass kernel: masked-centroid squared distances (8 NeuronCores, SPMD).

Reference computation (fp32):
    C = U^T X / B                          [K, D]   (B=512, K=512, D=1024)
    mask = round(clip(M, 0, 1)) = (M > 0.5)
    D_out[b, k] = sum_d mask[k,d] * (X[b,d] - C[k,d])^2

Algebraic expansion (mask^2 = mask):
    D_out[b,k] = sum_d mask*X^2  - 2*sum_d (mask*C)*X  + sum_d mask*C^2

Sharding: each of the 8 cores owns a 64-row shard of C / mask / D_out^T
(out_dim shard) -> every core needs full X but no collectives at all.

Per-core dataflow (d-major layout, d on partitions for the big matmuls):
    Ĉᵀraw[d,k] chunks = sum_b X[b,d] U_s[b,k]   (PE, fp8 ops, direct into the
        transposed layout: lhsT = X b-chunk, rhs = U_s b-chunk -> [128, 64])
    maskᵀ = (Mᵀ > 0.5)            (PE transpose of fp32 M, DVE is_gt -> bf16)
    CMᵀ  = (Ĉᵀraw * -1/256) * maskᵀ = -2*mask*C     (fused DVE stt -> bf16)
    Gᵀ   = (Ĉᵀraw * -1/256) * CMᵀ  = 4*mask*C^2    (fused DVE stt -> bf16)
    X2ᵀ  = XTᵀ * XTᵀ                                (DVE/ACT -> bf16)
    Dᵀ  += maskᵀ.T @ X2ᵀ   (T1)     (PE bf16, accum in PSUM [64, 512])
    Dᵀ  += CMᵀ.T  @ XTᵀ    (-2*T2)
    t3   = colsum(Gᵀ)/4 -> [1, 64]  (PE ones-colsum + ACT scale)
    Dᵀ  += t3 ⊗ 1   (rank-1 K=1 matmul closes the same PSUM group)
    Dᵀ -> SBUF (split DVE/ACT half-copies) -> DMA out [64, 512] f32

Precision: X enters the distance terms in bf16 (both layouts).  X and U enter
the *centroid* matmul in fp8e4m3 — C is ~40x smaller than X in magnitude and
only enters D through second-order terms, so fp8's ~4% element error adds
~1e-4 relative error to D while cutting the centroid operands' DMA 4x.
M stays fp32: the mask threshold (M > 0.5) must match the fp32 reference
bit-for-bit near 0.5.

The PE p-state ramps to full clock only after sustained busy (HAM clock
gate); warm-up/filler matmuls keep the PE dense through the real work.

Host does layout/dtype prep only (casts, transpose, shard, gather); all
FLOPs of the algorithm run on device.
"""

import numpy as np

BATCH = 512
OUT_DIM = 512
IN_DIM = 1024
N_CORES = 8
KS = OUT_DIM // N_CORES  # 64 centroid rows per core

_CACHE = {}


def build_module(num_devices: int = N_CORES):
    """Build + compile the Bass module (same SPMD program for every core)."""
    import concourse.bacc as bacc
    import concourse.mybir as mybir
    from concourse import tile
    from concourse.masks import make_identity

    if num_devices in _CACHE:
        return _CACHE[num_devices]

    fp32 = mybir.dt.float32
    bf16 = mybir.dt.bfloat16
    fp8 = mybir.dt.float8e4
    Alu = mybir.AluOpType
    Act = mybir.ActivationFunctionType

    nc = bacc.Bacc("TRN2", target_bir_lowering=False, debug=False,
                   num_devices=num_devices)

    NB = BATCH // 128   # 4 b-chunks
    ND = IN_DIM // 128  # 8 d-chunks

    # xb arrives d-chunk-major: xbj[m][p, 256*i + dd] = X[128*i + p, 256*m + dd]
    # so the j-major centroid accumulation groups pace with the DMA stream.
    xb = nc.dram_tensor("xb", [NB, 128, IN_DIM], fp8, kind="ExternalInput").ap()
    xt = nc.dram_tensor("xt", [IN_DIM, BATCH], bf16, kind="ExternalInput").ap()
    us = nc.dram_tensor("us", [BATCH, KS], fp8, kind="ExternalInput").ap()
    # mask source arrives pre-transposed+packed: ms[p, 64*j + k] = M_s[k, 128*j + p]
    ms = nc.dram_tensor("ms", [128, 512], fp32, kind="ExternalInput").ap()
    dt_out = nc.dram_tensor("dt", [KS, BATCH], fp32, kind="ExternalOutput").ap()

    with tile.TileContext(nc) as tc:
        with (
            tc.tile_pool(name="const", bufs=1) as constp,
            tc.tile_pool(name="xbp", bufs=1) as xbp,
            tc.tile_pool(name="xtp", bufs=1) as xtp,
            tc.tile_pool(name="x2tp", bufs=1) as x2tp,
            tc.tile_pool(name="smal", bufs=1) as smal,
            tc.tile_pool(name="psum", bufs=1, space="PSUM") as psp,
        ):
            # ---- constants
            ident = constp.tile([64, 64], fp32, tag="ident")
            make_identity(nc, ident[:, :])
            wtile = constp.tile([128, 512], bf16, tag="wtile")
            nc.gpsimd.memset(wtile[:, :], 0.0)
            ones_col = constp.tile([128, 1], bf16, tag="ones")
            nc.vector.memset(ones_col[:, :], 1.0)
            onesrow = constp.tile([1, 512], bf16, tag="onesrow")
            nc.vector.memset(onesrow[:, :], 1.0)

            # ---- DMA in.  HWDGE order = arrival order: mask source first
            # (feeds the longest latency ladder), then xt d-chunks 0/1 (T1
            # can start early), centroid xb halves, then the remaining xt
            # chunks, small ones last for a short post-stream tail.  The tiny
            # us tensor rides the parallel SWDGE (gpsimd) queue.
            ms_sb = smal.tile([128, 512], fp32, tag="ms")
            nc.sync.dma_start(ms_sb[:, :], ms[:, :])

            us_sb = smal.tile([128, NB * KS], fp8, tag="us")
            nc.gpsimd.dma_start(
                us_sb[:, :].rearrange("p (i k) -> p i k", i=NB),
                us.rearrange("(i p) k -> p i k", p=128),
            )

            xt_q = [xtp.tile([128, 2 * BATCH], bf16, tag=f"xtq{q}", name=f"xtq{q}")
                    for q in range(3)]
            xt_s = [xtp.tile([128, BATCH], bf16, tag=f"xts{j}", name=f"xts{j}")
                    for j in (6, 7)]

            def dma_xtq(q):
                nc.sync.dma_start(
                    xt_q[q][:, :].rearrange("p (r b) -> p r b", r=2),
                    xt[256 * q:256 * (q + 1), :].rearrange("(r p) b -> p r b", p=128),
                )

            dma_xtq(0)

            xb_t = [xbp.tile([128, 2 * IN_DIM], fp8, tag=f"xb{a}", name=f"xb{a}")
                    for a in range(2)]
            for a in range(2):
                nc.sync.dma_start(
                    xb_t[a][:, :].rearrange("p (m v) -> p m v", m=2),
                    xb[2 * a:2 * (a + 1), :, :].rearrange("m p v -> p m v"),
                )

            dma_xtq(1)
            dma_xtq(2)
            for idx, j in enumerate((6, 7)):
                nc.sync.dma_start(xt_s[idx][:, :], xt[128 * j:128 * (j + 1), :])

            def xt_slice(j):
                if j < 6:
                    return xt_q[j // 2][:, 512 * (j % 2):512 * (j % 2 + 1)]
                return xt_s[j - 6][:, :]

            # ---- PE warm-up: dummy matmuls (no data deps) ramp the PE clock
            # while DMAs land; they write psum_d which T1-j0 later resets.
            psum_d = psp.tile([64, 512], fp32, tag="pd")
            psum_w = psp.tile([64, 512], fp32, tag="pw")

            def dummy_mm(n=512):
                nc.tensor.matmul(psum_w[:, 0:n], wtile[:, 0:64], wtile[:, 0:n],
                                 start=True, stop=True)

            for _ in range(5):
                dummy_mm()

            # ---- maskᵀ = (Mᵀ > 0.5): Mᵀ arrives pre-packed from the host
            maskt = smal.tile([128, 512], bf16, tag="maskt")
            nc.vector.tensor_scalar(maskt[:, :], ms_sb[:, :], 0.5, None,
                                    Alu.is_gt)

            # ---- Ĉᵀraw[d,k] direct: per d-chunk j accumulate over b-chunks.
            # lhsT = X[b-chunk, d-chunk] (fp8), rhs = U_s[b-chunk] (fp8).
            # j-major (one pending PSUM accumulation group at a time); each
            # xb half covers 4 whole j-groups, so pacing is preserved.
            psum_ct = psp.tile([128, 512], fp32, tag="pct")
            for j in range(ND):
                a, mm = divmod(j, 4)  # xb half a, chunk-pair m'=mm//2, r=mm%2
                base = 1024 * (mm // 2) + 128 * (mm % 2)
                for i in range(NB):
                    nc.tensor.matmul(
                        psum_ct[:, 64 * j:64 * (j + 1)],
                        xb_t[a][:, base + 256 * i:base + 256 * i + 128],
                        us_sb[:, KS * i:KS * (i + 1)],
                        start=(i == 0), stop=(i == NB - 1),
                    )
                if j == 3:
                    dummy_mm()  # bridge the wait for the second xb half

            # ---- X2ᵀ squares as per-j [128, 512] units (each feeds exactly
            # one T1 matmul) alternating DVE/ACT, plus fused CM/G products.
            x2t_q = [x2tp.tile([128, 2 * BATCH], bf16, tag=f"x2q{q}", name=f"x2q{q}")
                     for q in range(3)]
            x2t_s = [x2tp.tile([128, BATCH], bf16, tag=f"x2s{j}", name=f"x2s{j}")
                     for j in (6, 7)]

            def x2t_slice(j):
                if j < 6:
                    return x2t_q[j // 2][:, 512 * (j % 2):512 * (j % 2 + 1)]
                return x2t_s[j - 6][:, :]

            SQ_ON_ACT = {1, 2, 4, 6}
            for j in range(ND):
                dst, srcap = x2t_slice(j), xt_slice(j)
                if j in SQ_ON_ACT:
                    nc.scalar.activation(dst, srcap, Act.Square)
                else:
                    nc.vector.tensor_tensor(dst, srcap, srcap, Alu.mult)

            cmt = smal.tile([128, 512], bf16, tag="cmt")
            nc.vector.scalar_tensor_tensor(cmt[:, :], psum_ct[:, :], -1.0 / 256.0,
                                           maskt[:, :], Alu.mult, Alu.mult)
            g_sb = smal.tile([128, 512], bf16, tag="g")
            nc.vector.scalar_tensor_tensor(g_sb[:, :], psum_ct[:, :], -1.0 / 256.0,
                                           cmt[:, :], Alu.mult, Alu.mult)

            # ---- Dᵀ accumulation, interleaved to match operand arrival
            def t1(j, start=False):
                nc.tensor.matmul(psum_d[:, :], maskt[:, 64 * j:64 * (j + 1)],
                                 x2t_slice(j), start=start, stop=False)

            def t2(j):
                nc.tensor.matmul(psum_d[:, :], cmt[:, 64 * j:64 * (j + 1)],
                                 xt_slice(j), start=False, stop=False)

            t1(0, start=True)
            t1(1)
            dummy_mm(128)
            t2(0)
            t2(1)
            t2(2)
            t1(2)
            t1(3)
            dummy_mm(128)
            t2(3)
            t2(4)
            t1(4)
            t1(5)
            t2(5)

            # t3 colsums (ready with g; fill the wait for xt6/7)
            psum_t3 = psp.tile([1, 64], fp32, tag="pt3")
            for j in range(ND):
                nc.tensor.matmul(psum_t3[:, :], ones_col[:, :],
                                 g_sb[:, 64 * j:64 * (j + 1)],
                                 start=(j == 0), stop=(j == ND - 1))
            t3f = smal.tile([1, 64], bf16, tag="t3f")
            nc.scalar.activation(t3f[:, :], psum_t3[:, :], Act.Copy, scale=0.25)

            t1(6)
            t2(6)
            t1(7)
            t2(7)
            nc.tensor.matmul(psum_d[:, :], t3f[:, :], onesrow[:, :],
                             start=False, stop=True)

            # ---- Dᵀ -> SBUF (two parallel half-copies) -> DRAM
            d_sb = smal.tile([64, 512], fp32, tag="d")
            nc.vector.tensor_copy(d_sb[:, :], psum_d[:, :])
            nc.sync.dma_start(dt_out[:, :], d_sb[:, :])

    nc.compile()
    _CACHE[num_devices] = nc
    return nc


def kernel(X: np.ndarray, U: np.ndarray, M: np.ndarray) -> np.ndarray:
    import ml_dtypes
    from concourse import bass_utils

    nc = build_module(N_CORES)

    bf16 = ml_dtypes.bfloat16
    fp8 = ml_dtypes.float8_e4m3
    # d-chunk-major fp8 layout: xbj[m][p, 256*i + dd] = X[128*i + p, 256*m + dd]
    xb_np = np.ascontiguousarray(
        X.reshape(4, 128, 4, 256).transpose(2, 1, 0, 3).reshape(4, 128, 1024)
    ).astype(fp8)
    xt_np = np.ascontiguousarray(X.T).astype(bf16)

    in_maps = []
    for c in range(N_CORES):
        in_maps.append({
            "xb": xb_np,
            "xt": xt_np,
            "us": np.ascontiguousarray(U[:, KS * c:KS * (c + 1)]).astype(fp8),
            "ms": np.ascontiguousarray(
                M[KS * c:KS * (c + 1), :].T.reshape(8, 128, KS)
                .transpose(1, 0, 2).reshape(128, 512)).astype(np.float32),
        })

    res = bass_utils.run_bass_kernel_spmd(nc, in_maps, core_ids=list(range(N_CORES)))

    out = np.empty((BATCH, OUT_DIM), dtype=np.float32)
    for c in range(N_CORES):
        out[:, KS * c:KS * (c + 1)] = res.results[c]["dt"].T
    return out
